# revision 29
# baseline (speedup 1.0000x reference)
"""Trainium2 Bass kernel for a 6-layer GCN autoencoder (50k nodes, 800k edges).

Self-contained: host-side graph preprocessing (node permutation/packing,
edge binning, degree norms), an 8-core SPMD Bass/Tile program (gather-first
dataflow, one-hot scatter matmuls, dma_gather striped over all 4 SWDGE
queues, AllGather collectives), and output assembly.

Layer plumbing (per-node norm d = deg^-1/2, s = deg^1/2, so d*s = 1):
  L1 gathers host-staged xs = d*x directly (no transform, no AllGather):
      a1 = d * (sum_in xs[src] + xs[self])            == Ahat x
      u2 = relu(eW1^T a1 + eb1)                       == h1 (exact)
  L2: h2' = d*(u2 eW2p) node-major -> AllGather -> spmm with dst-side
      sqd x eb2p bias -> w2 (raw);   z-lite: zrow = d^2*(w2 eWfp + s x ebf)
      == d*z node-major -> AllGather
  L3: a3 = d * (sum zrow[src] + zrow[self]); u4 = relu(dW1p^T a3 + db1) == h3
  L4: h4' = d*(u4 dW2) -> AllGather -> spmm w/ sqd x db2 -> f (raw)
  out: xhat = d*(f dWf + s x dbf), written bf16, host-cast to f32.
"""
import sys
sys.path.insert(0, '/opt/trn_rl_repo')

import contextlib
import ctypes
import os
import types

import numpy as np
import ml_dtypes

import concourse.bacc as bacc
import concourse.bass as bass
import concourse.mybir as mybir
import concourse.tile as tile
from concourse.library_config import mlp
from concourse.vector_clock import ScopedClock
from concourse.bass_utils import run_bass_kernel_spmd


# ---- workaround: this walrus build rejects >2 sync waits on one instruction;
# spread Tile's tail-drain waits across single-wait SP NOPs.
def _patched_drain_and_barrier(self, tick_clock, wait_clock):
    nc = self.nc
    probe = nc.sync.nop()
    wait_clock.add_sem_waits(probe.ins, ScopedClock({None: tick_clock.global_clock}))
    si = probe.ins.sync_info
    waits = list(si.on_wait) if si is not None else []
    if si is not None:
        while si.on_wait:
            si.on_wait.pop()
    for w in waits:
        n = nc.sync.nop()
        n.ins.sync_info = mybir.SyncInfo(on_wait=[w], on_update=[])
    nc.sync.drain()
    nc.all_engine_barrier()
    assert self.sems is not None
    popped = nc._tile_sem_poison_stack.pop()
    assert popped is self._sem_poison
    nc.clear_and_free_semaphores(list(self.sems.allocated().values()))
    nc.all_engine_barrier()


tile.TileContext._drain_and_barrier = _patched_drain_and_barrier


# ---- optional NTFF profiling hook (GCAE_TRACE=1)
def _install_profile_hook():
    try:
        import antenv
    except ImportError:
        return False
    if getattr(antenv, "axon_hooks", None) is not None:
        return True
    so_path = "/opt/axon/libaxon_pjrt.so"
    if not os.path.exists(so_path):
        return False
    lib = ctypes.CDLL(so_path)
    if not hasattr(lib, "axon_start_nrt_profile"):
        return False
    lib.axon_start_nrt_profile.argtypes = [ctypes.POINTER(ctypes.c_int64), ctypes.c_size_t]
    lib.axon_start_nrt_profile.restype = ctypes.c_int64
    lib.axon_stop_nrt_profile.argtypes = [ctypes.c_char_p]
    lib.axon_stop_nrt_profile.restype = ctypes.c_int64

    @contextlib.contextmanager
    def _hook(output_dir, device_ids):
        import jax
        jax.devices()
        if device_ids:
            ids = (ctypes.c_int64 * len(device_ids))(*device_ids)
            rc = lib.axon_start_nrt_profile(ids, len(device_ids))
        else:
            rc = lib.axon_start_nrt_profile(None, 0)
        if rc != 0:
            raise RuntimeError(f"axon_start_nrt_profile rc={rc}")
        try:
            yield
        finally:
            n = lib.axon_stop_nrt_profile(str(output_dir).encode())
            if n < 0:
                raise RuntimeError(f"axon_stop_nrt_profile rc={n}")

    hooks = types.ModuleType("antenv.axon_hooks")
    _h = [_hook]
    hooks.set_axon_ntff_profile_hook = lambda h: _h.__setitem__(0, h)
    hooks.get_axon_ntff_profile_hook = lambda: _h[0]
    sys.modules["antenv.axon_hooks"] = hooks
    antenv.axon_hooks = hooks
    return True

F32 = mybir.dt.float32
BF16 = mybir.dt.bfloat16
I16 = mybir.dt.int16

N = 50000
NC = 8
BLK = 128
BPC = 49                 # blocks per core
NPC = BPC * BLK          # 6272 nodes per core
NPAD = NC * NPC          # 50176
HALF = NPAD // 2         # 25088
NT = BPC                 # node tiles per core
LOB = 24                 # blocks per core in sub-shard A
LOA = LOB * BLK          # 3072 rows/core in sub-shard A
HIB = NPC - LOA          # 3200 rows/core in sub-shard B


# ---------------------------------------------------------------- host prep

def preprocess(edge_index):
    src = np.asarray(edge_index[0], dtype=np.int64)
    dst = np.asarray(edge_index[1], dtype=np.int64)

    # degree includes the self-loops even though they are not in the edge
    # stream (they are applied on-device via an identity matmul)
    deg = (np.bincount(dst, minlength=N) + 1).astype(np.float64)
    dinv = np.where(deg > 0, 1.0 / np.sqrt(deg), 0.0)
    sqrtdeg = np.where(deg > 0, np.sqrt(deg), 0.0)

    # snake-deal nodes (sorted by degree desc) into 392 blocks
    nblocks = NC * BPC
    order = np.argsort(-deg, kind="stable")
    node_new = np.empty(N, dtype=np.int64)
    counts = np.zeros(nblocks, dtype=np.int64)
    bi = 0
    direction = 1
    for i, nd in enumerate(order):
        b = bi if direction == 1 else nblocks - 1 - bi
        node_new[nd] = b * BLK + counts[b]
        counts[b] += 1
        bi += 1
        if bi == nblocks:
            bi = 0
            direction = -direction
    assert counts.max() <= BLK

    # Edge halves (A/B sub-shard of the SRC) are fixed by which sub-shard a
    # node sits in; repacking nodes WITHIN a (core, sub-shard) never flips
    # any edge's half. Two host-only refinements under that invariant:
    # 1) 2D repack: redistribute nodes among their sub-shard's blocks to
    #    balance per-block (lo, hi) in-edge counts.
    # 2) slot alignment: within each sub-shard, order each core's blocks by
    #    lo-count so slot b holds similarly-sized groups on every core (the
    #    shared tile schedule takes max-over-cores per slot).
    s_p0 = node_new[src]
    d_p0 = node_new[dst]
    hf0 = ((s_p0 % NPC) >= LOA).astype(np.int64)
    node_lo = np.bincount(d_p0[hf0 == 0], minlength=NPAD).astype(np.int64)
    node_hi = np.bincount(d_p0[hf0 == 1], minlength=NPAD).astype(np.int64)

    new_pos = np.empty(NPAD, dtype=np.int64)
    for c in range(NC):
        for sh, (b0, nb) in enumerate(((0, LOB), (LOB, BPC - LOB))):
            rows = np.arange(c * NPC + b0 * BLK, c * NPC + (b0 + nb) * BLK)
            lo, hi = node_lo[rows], node_hi[rows]
            order = np.argsort(-(lo + hi), kind="stable")
            losum = np.zeros(nb); hisum = np.zeros(nb)
            nfill = np.zeros(nb, dtype=np.int64)
            assign = np.empty(len(rows), dtype=np.int64)
            for i in order:
                cost = (losum + lo[i]) ** 2 + (hisum + hi[i]) ** 2
                cost[nfill >= BLK] = np.inf
                j = int(np.argmin(cost))
                assign[i] = j
                losum[j] += lo[i]; hisum[j] += hi[i]; nfill[j] += 1
            slot_of = np.empty(nb, dtype=np.int64)
            slot_of[np.argsort(-losum, kind="stable")] = np.arange(nb)
            fill2 = np.zeros(nb, dtype=np.int64)
            for i in range(len(rows)):
                j = slot_of[assign[i]]
                new_pos[rows[i]] = c * NPC + (b0 + j) * BLK + fill2[j]
                fill2[j] += 1
    node_new = new_pos[node_new]

    s_p = node_new[src]
    d_p = node_new[dst]

    # per (core, block, half) edge lists
    core = d_p // NPC
    blk = (d_p % NPC) // BLK
    dloc = d_p % BLK
    score = s_p // NPC
    w = s_p % NPC
    hf = (w >= LOA).astype(np.int64)
    idxh = np.where(hf == 0, score * LOA + w, score * HIB + (w - LOA))

    cnt = np.zeros((NC, BPC, 2), dtype=np.int64)
    np.add.at(cnt, (core, blk, hf), 1)
    T = np.maximum(1, np.ceil(cnt.max(axis=0) / BLK).astype(np.int64))  # [BPC, 2]

    offs = np.zeros((BPC, 2), dtype=np.int64)   # tile offset of each (b, hf) group
    t = 0
    for b in range(BPC):
        for h in range(2):
            offs[b, h] = t
            t += T[b, h]
    TT = t                                       # total tiles per core

    idx_all = np.full((NC, TT * BLK), -1, dtype=np.int16)
    dloc_all = np.full((NC, TT * BLK), -1.0, dtype=np.float32)
    key = (core * BPC + blk) * 2 + hf
    ordkey = np.lexsort((idxh, key))
    ks = key[ordkey]
    sc, sb, sh = core[ordkey], blk[ordkey], hf[ordkey]
    si, sd = idxh[ordkey], dloc[ordkey]
    ne = len(ks)
    starts = np.r_[0, np.flatnonzero(np.diff(ks)) + 1]
    glen = np.diff(np.r_[starts, ne])
    pos = np.arange(ne) - np.repeat(starts, glen)
    slot = offs[sb, sh] * BLK + pos
    idx_all[sc, slot] = si.astype(np.int16)
    dloc_all[sc, slot] = sd.astype(np.float32)

    sd_pad = np.zeros(NPAD, dtype=np.float32)
    di_pad = np.zeros(NPAD, dtype=np.float32)
    sd_pad[node_new] = sqrtdeg
    di_pad[node_new] = dinv

    return dict(node_new=node_new, T=T, offs=offs, TT=TT, cnt=cnt,
                idx_all=idx_all, dloc_all=dloc_all,
                sqrtdeg=sd_pad, dinv=di_pad)


def make_inmaps(pre, x, weights):
    """weights: dict of padded bf16 weight/bias arrays (shared across cores)."""
    node_new = pre["node_new"]
    TT = pre["TT"]
    Tmax = int(pre["T"].max())
    bf = ml_dtypes.bfloat16

    # host-staged, dinv-prescaled node-major x (the L1 gather source)
    xs = np.zeros((NPAD, 128), dtype=np.float32)
    xs[node_new] = np.asarray(x, dtype=np.float32)
    xs *= pre["dinv"][:, None]
    xs3 = xs.reshape(NC, NPC, 128)
    xsA = np.ascontiguousarray(xs3[:, :LOA, :].reshape(NC * LOA, 128)).astype(bf)
    xsB = np.ascontiguousarray(xs3[:, LOA:, :].reshape(NC * HIB, 128)).astype(bf)

    in_maps = []
    for c in range(NC):
        m = {}
        m["xsA"] = xsA
        m["xsB"] = xsB
        m["xloc"] = np.ascontiguousarray(xs3[c]).astype(bf)    # [NPC, 128]
        idx = pre["idx_all"][c]
        m["idxs"] = np.tile(idx.reshape(TT * 8, 16).T, (8, 1)).copy()
        # host-precomputed one-hot scatter matrices (identical for all four
        # layers): Sbig[p, t*128+d] = 1.0 iff dloc[t*128+p] == d
        dl2 = pre["dloc_all"][c].reshape(TT, BLK)
        S = (dl2[:, :, None] == np.arange(128, dtype=np.float32)).astype(bf)
        m["Sbig"] = np.ascontiguousarray(S.transpose(1, 0, 2).reshape(BLK, TT * 128))
        sl = slice(c * NPC, (c + 1) * NPC)
        gc = pre["cnt"][c].T.reshape(1, 2 * BPC)               # [1, 2*BPC], hf-major
        m["gcount"] = np.ascontiguousarray(gc, dtype=np.int32)
        m["sqrtdeg_row"] = pre["sqrtdeg"][sl][None, :].astype(bf)
        m["dinv_col"] = pre["dinv"][sl].reshape(BPC, BLK).T.astype(np.float32).copy()
        m["dinv2_col"] = (pre["dinv"][sl] ** 2).reshape(BPC, BLK).T.astype(np.float32).copy()
        m["dinvb"] = np.tile(pre["dinv"][sl][None, :], (128, 1)).astype(bf)
        m["one_row"] = np.ones((1, 128), dtype=np.float32).astype(bf)
        m["ident"] = np.eye(128, dtype=np.float32).astype(bf)
        m.update(weights)
        in_maps.append(m)
    return in_maps


def pad_weights(eW1, eb1, eW2, eb2, eWf, ebf, dW1, db1, dW2, db2, dWf, dbf):
    bf = ml_dtypes.bfloat16
    w = {}
    w["eW1"] = np.asarray(eW1, np.float32).astype(bf)                       # [128,128]
    eW2p = np.zeros((128, 128), np.float32); eW2p[:, :64] = eW2
    w["eW2p"] = eW2p.astype(bf)
    eWfp = np.zeros((128, 128), np.float32); eWfp[:64, :64] = eWf
    w["eWfp"] = eWfp.astype(bf)                                             # [128,128]
    dW1p = np.zeros((128, 256), np.float32); dW1p[:64] = dW1
    w["dW1p"] = dW1p.astype(bf)                                             # [128,256]
    w["dW2"] = np.asarray(dW2, np.float32).astype(bf)                       # [256,128]
    w["dWf"] = np.asarray(dWf, np.float32).astype(bf)                       # [128,1024]
    w["eb1_row"] = np.asarray(eb1, np.float32)[None, :].astype(bf)          # [1,128]
    eb2r = np.zeros((1, 128), np.float32); eb2r[0, :64] = eb2
    w["eb2p_row"] = eb2r.astype(bf)
    ebfr = np.zeros((1, 128), np.float32); ebfr[0, :64] = ebf
    w["ebf_row"] = ebfr.astype(bf)                                          # [1,128]
    w["db1_row"] = np.asarray(db1, np.float32)[None, :].astype(bf)          # [1,256]
    w["db2_row"] = np.asarray(db2, np.float32)[None, :].astype(bf)          # [1,128]
    w["dbf_row"] = np.asarray(dbf, np.float32)[None, :].astype(bf)          # [1,1024]
    return w


# ---------------------------------------------------------------- device program

def build_program(T, offs, TT):
    """T: [BPC,2] tiles per (block,half); offs: [BPC,2] tile offsets; TT total tiles."""
    Tmax = int(T.max())
    nc = bacc.Bacc(None, target_bir_lowering=False, num_swdge_queues=4)

    # ---- I/O
    xsA_d = nc.dram_tensor("xsA", [NC * LOA, 128], BF16, kind="ExternalInput")
    xsB_d = nc.dram_tensor("xsB", [NC * HIB, 128], BF16, kind="ExternalInput")
    xloc_d = nc.dram_tensor("xloc", [NPC, 128], BF16, kind="ExternalInput")
    idx_d = nc.dram_tensor("idxs", [128, TT * 8], I16, kind="ExternalInput")
    S_d = nc.dram_tensor("Sbig", [128, TT * 128], BF16, kind="ExternalInput")
    gcount_d = nc.dram_tensor("gcount", [1, 2 * BPC], mybir.dt.int32, kind="ExternalInput")
    sqd_d = nc.dram_tensor("sqrtdeg_row", [1, NPC], BF16, kind="ExternalInput")
    dinv_d = nc.dram_tensor("dinv_col", [128, BPC], F32, kind="ExternalInput")
    dinv2_d = nc.dram_tensor("dinv2_col", [128, BPC], F32, kind="ExternalInput")
    dinvb_d = nc.dram_tensor("dinvb", [128, NPC], BF16, kind="ExternalInput")
    one_d = nc.dram_tensor("one_row", [1, 128], BF16, kind="ExternalInput")
    id_d = nc.dram_tensor("ident", [128, 128], BF16, kind="ExternalInput")
    wnames = {"eW1": [128, 128], "eW2p": [128, 128], "eWfp": [128, 128],
              "dW1p": [128, 256], "dW2": [256, 128], "dWf": [128, 1024],
              "eb1_row": [1, 128], "eb2p_row": [1, 128], "ebf_row": [1, 128],
              "db1_row": [1, 256], "db2_row": [1, 128], "dbf_row": [1, 1024]}
    w_d = {k: nc.dram_tensor(k, shp, BF16, kind="ExternalInput")
           for k, shp in wnames.items()}
    out_d = nc.dram_tensor("xhat", [2, NPC, 512], BF16, kind="ExternalOutput")

    with tile.TileContext(nc) as tc:
        with tc.tile_pool(name="const", bufs=1) as cpool, \
             tc.tile_pool(name="acts", bufs=1) as apool, \
             tc.tile_pool(name="dram", bufs=1, space="DRAM") as dram, \
             tc.tile_pool(name="wps", bufs=4, space="PSUM") as pps, \
             tc.tile_pool(name="wtr", bufs=4, space="PSUM") as ptr, \
             tc.tile_pool(name="wm", bufs=8) as pm, \
             tc.tile_pool(name="ws", bufs=4) as psl, \
             tc.tile_pool(name="wh", bufs=3) as ph, \
             tc.tile_pool(name="wn", bufs=4) as phn:
            nc.gpsimd.load_library(mlp)

            # ---- persistent SBUF state (gather-critical loads first: the
            # sync engine issues DMAs in order and the first dma_gather only
            # needs gcount + idx)
            gcount_sb = cpool.tile([1, 2 * BPC], mybir.dt.int32, name="gcount_sb")
            nc.sync.dma_start(gcount_sb[:], gcount_d[:])
            idx_sb = cpool.tile([128, TT * 8], I16, name="idx_sb")
            nc.sync.dma_start(idx_sb[:], idx_d[:])
            id_sb = cpool.tile([128, 128], BF16, name="id_sb")
            nc.sync.dma_start(id_sb[:], id_d[:])
            w_sb = {}
            for k, shp in wnames.items():
                if shp[0] > 128:
                    continue
                t = cpool.tile(shp, BF16, name=f"w_{k}")
                nc.sync.dma_start(t[:], w_d[k][:])
                w_sb[k] = t
            dW2a = cpool.tile([128, 128], BF16, name="w_dW2a")
            nc.sync.dma_start(dW2a[:], w_d["dW2"][0:128, :])
            dW2b = cpool.tile([128, 128], BF16, name="w_dW2b")
            nc.sync.dma_start(dW2b[:], w_d["dW2"][128:256, :])
            sqd_sb = cpool.tile([1, NPC], BF16, name="sqd_sb")
            nc.sync.dma_start(sqd_sb[:], sqd_d[:])
            dinv_sb = cpool.tile([128, BPC], F32, name="dinv_sb")
            nc.sync.dma_start(dinv_sb[:], dinv_d[:])
            dinv2_sb = cpool.tile([128, BPC], F32, name="dinv2_sb")
            nc.sync.dma_start(dinv2_sb[:], dinv2_d[:])
            dinvb_sb = cpool.tile([128, NPC], BF16, name="dinvb_sb")
            nc.sync.dma_start(dinvb_sb[:], dinvb_d[:])
            one_sb = cpool.tile([1, 128], BF16, name="one_sb")
            nc.sync.dma_start(one_sb[:], one_d[:])

            uT = {}  # feature-major activation arrays per stage
            for nm in ("a1", "u2", "w2", "a3", "u4a", "u4b", "f"):
                uT[nm] = apool.tile([128, NPC], BF16, name=f"{nm}T")

            qstate = [0]

            # ---------------- SpMM: gather-aggregate into per-block PSUM
            # chains; pass A includes the self-loop identity mm (+ optional
            # dst-side sqd x bias mm); pass B re-injects the bf16 pass-A
            # partial via an identity matmul; per-block copy-out via copy_cb.
            def spmm(layer, bufA, bufB, locA, locB, bias_row, out_t, copy_cb,
                     epi=None, border=None):
                for hf in range(2):
                    buf = bufA if hf == 0 else bufB
                    blks = range(BPC) if (hf == 0 or border is None) else border
                    for b in blks:
                        Tb = int(T[b, hf]); off = int(offs[b, hf])
                        pb = pps.tile([128, 128], F32, tag="pb")
                        if hf == 0:
                            hblk = ph.tile([128, 128], BF16, tag="hblk")
                            if b < LOB:
                                nc.sync.dma_start(hblk[:], locA[b * 128:(b + 1) * 128, :])
                            else:
                                nc.sync.dma_start(hblk[:], locB[(b - LOB) * 128:(b - LOB + 1) * 128, :])
                            if bias_row is not None:
                                nc.tensor.matmul(
                                    pb[:], bias_row[0:1, :],
                                    sqd_sb[0:1, b * 128:(b + 1) * 128],
                                    start=True, stop=False)
                            nc.tensor.matmul(
                                pb[:], hblk[:], id_sb[:],
                                start=(bias_row is None), stop=False)
                        else:
                            nc.tensor.matmul(
                                pb[:], id_sb[:], out_t[:, b * 128:(b + 1) * 128],
                                start=True, stop=False)
                        msg = pm.tile([128, Tmax, 128], BF16, tag="msg")
                        if layer == 1 and hf == 0 and b < 8:
                            nc.vector.memset(msg[:], 0.0)
                        creg = nc.gpsimd.alloc_register()
                        nc.gpsimd.load(creg, gcount_sb[0:1, hf * BPC + b:hf * BPC + b + 1])
                        nc.gpsimd.dma_gather(
                            msg[:, :Tb, :], buf[:],
                            idx_sb[:, off * 8:(off + Tb) * 8],
                            Tb * 128, creg, 128, single_packet=False,
                            queue_num=qstate[0])
                        qstate[0] = (qstate[0] + 1) % 4
                        # scatter one-hots are host-precomputed; load via the
                        # ACT engine's HWDGE queue (DMA-side SBUF ports don't
                        # contend with the Q7 descriptor-gen engine ports)
                        S = psl.tile([128, Tmax * 128], BF16, tag="S")
                        nc.scalar.dma_start(S[:, 0:Tb * 128],
                                            S_d[:, off * 128:(off + Tb) * 128])
                        for t in range(Tb):
                            nc.tensor.matmul(
                                pb[:], msg[:, t, :], S[:, t * 128:(t + 1) * 128],
                                start=False, stop=(t == Tb - 1))
                        osl = out_t[:, b * 128:(b + 1) * 128]
                        copy_cb(b, hf, pb, osl)
                        if epi is not None and hf == 1:
                            epi(b)

            def cb_plain(b, hf, pb, osl):
                nc.scalar.activation(osl, pb[:], mybir.ActivationFunctionType.Copy)

            def cb_dinvb(b, hf, pb, osl):
                if hf == 0:
                    nc.scalar.activation(osl, pb[:],
                                         mybir.ActivationFunctionType.Copy)
                else:
                    nc.vector.tensor_tensor(
                        osl, pb[:], dinvb_sb[:, b * 128:(b + 1) * 128],
                        mybir.AluOpType.mult)

            def cb_final(b, hf, pb, osl):
                nc.scalar.activation(osl, pb[:], mybir.ActivationFunctionType.Copy)
                if hf == 0:
                    return
                # final stage for block b: xhat = d*(f dWf + s x dbf), bf16 out
                for cb in range(2):
                    pf = ptr.tile([128, 512], F32, tag="tr")
                    nc.tensor.matmul(pf[:], osl,
                                     w_sb["dWf"][:, cb * 512:(cb + 1) * 512],
                                     start=True, stop=False)
                    nc.tensor.matmul(pf[:],
                                     sqd_sb[0:1, b * 128:(b + 1) * 128],
                                     w_sb["dbf_row"][0:1, cb * 512:(cb + 1) * 512],
                                     start=False, stop=True)
                    ob = phn.tile([128, 512], BF16, tag="ob")
                    nc.scalar.activation(ob[:], pf[:],
                                         mybir.ActivationFunctionType.Copy,
                                         scale=dinv_sb[:, b:b + 1])
                    nc.sync.dma_start(out_d[cb, b * 128:(b + 1) * 128, :], ob[:])

            # ---------------- feature-major "lite" transform, one 128-node tile:
            # out_fm = act(W^T @ in_fm + bias x ones)
            def tlite_tile(nt, in_t, Ws, bias_row, out_ts, act):
                for chb in range(len(out_ts)):
                    pt = ptr.tile([128, 128], F32, tag="tr")
                    nc.tensor.matmul(pt[:], Ws[:, chb * 128:(chb + 1) * 128],
                                     in_t[:, nt * 128:(nt + 1) * 128],
                                     start=True, stop=False)
                    nc.tensor.matmul(pt[:], bias_row[0:1, chb * 128:(chb + 1) * 128],
                                     one_sb[0:1, :], start=False, stop=True)
                    nc.scalar.activation(
                        out_ts[chb][:, nt * 128:(nt + 1) * 128], pt[:], act)

            # ---------------- node-major transform + shard write, one tile:
            # shard rows = scale_col * (sum_k u_k^T @ W_k [+ s x bias])
            def transform_tile(nt, parts, bias_row, shards, scale_col):
                shA, shB = shards
                hb = ptr.tile([128, 128], F32, tag="tr")
                for ki, (ut, Wk) in enumerate(parts):
                    nc.tensor.matmul(hb[:], ut[:, nt * 128:(nt + 1) * 128],
                                     Wk[:], start=(ki == 0),
                                     stop=(bias_row is None and
                                           ki == len(parts) - 1))
                if bias_row is not None:
                    nc.tensor.matmul(hb[:], sqd_sb[0:1, nt * 128:(nt + 1) * 128],
                                     bias_row[0:1, :], start=False, stop=True)
                hn = phn.tile([128, 128], BF16, tag="hn")
                nc.scalar.activation(hn[:], hb[:],
                                     mybir.ActivationFunctionType.Copy,
                                     scale=scale_col[:, nt:nt + 1])
                if nt < LOB:
                    nc.sync.dma_start(shA[nt * 128:(nt + 1) * 128, :], hn[:])
                else:
                    nc.sync.dma_start(shB[(nt - LOB) * 128:(nt - LOB + 1) * 128, :], hn[:])

            def mkshard(name, ch):
                sA = dram.tile([LOA, ch], BF16, name=f"{name}_shardA")
                sB = dram.tile([HIB, ch], BF16, name=f"{name}_shardB")
                fA = dram.tile([NC * LOA, ch], BF16, name=f"{name}_fullA", addr_space="Shared")
                fB = dram.tile([NC * HIB, ch], BF16, name=f"{name}_fullB", addr_space="Shared")
                return sA, sB, fA, fB

            def allgather(sX, fX):
                nc.gpsimd.collective_compute(
                    "AllGather", mybir.AluOpType.bypass,
                    replica_groups=[list(range(NC))],
                    ins=[sX.opt()], outs=[fX.opt()])

            # ================= network =================
            h2sA, h2sB, h2fA, h2fB = mkshard("h2", 128)
            h3sA, h3sB, h3fA, h3fB = mkshard("h3", 128)
            h4sA, h4sB, h4fA, h4fB = mkshard("h4", 128)

            relu_act = mybir.ActivationFunctionType.Relu

            # L1: gather host-staged xs; a1 = d*(sum + self); per-block
            # epilogue: u2 tile = relu(eW1^T a1 + eb1) == h1, then
            # h2' tile = d*(u2 eW2p) -> shard
            def epi1(b):
                tlite_tile(b, uT["a1"], w_sb["eW1"], w_sb["eb1_row"],
                           [uT["u2"]], relu_act)
                transform_tile(b, [(uT["u2"], w_sb["eW2p"])], None,
                               (h2sA[:], h2sB[:]), dinv_sb)

            spmm(1, xsA_d, xsB_d, xloc_d[0:LOA, :], xloc_d[LOA:NPC, :],
                 None, uT["a1"], cb_dinvb, epi=epi1)
            allgather(h2sA, h2fA)
            allgather(h2sB, h2fB)

            # L2 spmm (+eb2p); epilogue: zrow tile = d^2*(w2 eWfp + s x ebf)
            def epi2(b):
                transform_tile(b, [(uT["w2"], w_sb["eWfp"])], w_sb["ebf_row"],
                               (h3sA[:], h3sB[:]), dinv2_sb)

            spmm(2, h2fA, h2fB, h2sA[:], h2sB[:], w_sb["eb2p_row"], uT["w2"],
                 cb_plain, epi=epi2)
            allgather(h3sA, h3fA)
            allgather(h3sB, h3fB)

            # L3: a3 = d*(sum zrow + self); epilogue: u4 = relu(dW1p^T a3 +
            # db1) == h3, then h4' tile = d*(u4 dW2) -> shard
            def epi3(b):
                tlite_tile(b, uT["a3"], w_sb["dW1p"], w_sb["db1_row"],
                           [uT["u4a"], uT["u4b"]], relu_act)
                transform_tile(b, [(uT["u4a"], dW2a), (uT["u4b"], dW2b)], None,
                               (h4sA[:], h4sB[:]), dinv_sb)

            spmm(3, h3fA, h3fB, h3sA[:], h3sB[:], None, uT["a3"], cb_dinvb,
                 epi=epi3)
            allgather(h4sA, h4fA)
            allgather(h4sB, h4fB)

            # L4 spmm (+db2); final stage emitted per block via cb_final.
            # B-pass in descending group size so the post-stream tail chain
            # (last gather -> matmuls -> final -> DMA) is minimal.
            l4order = sorted(range(BPC), key=lambda b: -int(T[b, 1]))
            spmm(4, h4fA, h4fB, h4sA[:], h4sB[:], w_sb["db2_row"], uT["f"],
                 cb_final, border=l4order)

    nc.finalize()
    return nc


# ---------------------------------------------------------------- entry point

def kernel(x, edge_index, eW1, eb1, eW2, eb2, eWf, ebf,
           dW1, db1, dW2, db2, dWf, dbf):
    x = np.asarray(x, dtype=np.float32)
    edge_index = np.asarray(edge_index)

    pre = preprocess(edge_index)
    w = pad_weights(eW1, eb1, eW2, eb2, eWf, ebf, dW1, db1, dW2, db2, dWf, dbf)
    in_maps = make_inmaps(pre, x, w)
    nc = build_program(pre["T"], pre["offs"], pre["TT"])

    trace = os.environ.get("GCAE_TRACE", "0") == "1"
    if trace:
        trace = _install_profile_hook()
    res = None
    last_err = None
    for attempt in range(3):
        try:
            res = run_bass_kernel_spmd(nc, in_maps, core_ids=list(range(NC)),
                                       trace=trace and attempt == 0)
            break
        except Exception as e:  # transient device wedge: retry, drop tracing
            last_err = e
    if res is None:
        raise last_err
    if trace and res.exec_time_ns:
        print(f"HW exec time: {res.exec_time_ns} ns")

    xhat_pad = np.empty((NPAD, 1024), dtype=np.float32)
    for c in range(NC):
        o = np.asarray(res.results[c]["xhat"]).astype(np.float32)
        xhat_pad[c * NPC:(c + 1) * NPC, 0:512] = o[0]
        xhat_pad[c * NPC:(c + 1) * NPC, 512:1024] = o[1]
    return xhat_pad[pre["node_new"]]


# revision 36
# speedup vs baseline: 1.1603x; 1.1603x over previous
"""Trainium2 Bass kernel for a 6-layer GCN autoencoder (50k nodes, 800k edges).

Self-contained: host-side graph preprocessing (node permutation/packing,
edge binning, degree norms), an 8-core SPMD Bass/Tile program (gather-first
dataflow, one-hot scatter matmuls, dma_gather striped over all 4 SWDGE
queues, AllGather collectives), and output assembly.

Layer plumbing (per-node norm d = deg^-1/2, s = deg^1/2, so d*s = 1):
  L1 gathers host-staged xs = d*x directly (no transform, no AllGather):
      a1 = d * (sum_in xs[src] + xs[self])            == Ahat x
      u2 = relu(eW1^T a1 + eb1)                       == h1 (exact)
  L2: h2' = d*(u2 eW2p) node-major -> AllGather -> spmm with dst-side
      sqd x eb2p bias -> w2 (raw);   z-lite: zrow = d^2*(w2 eWfp + s x ebf)
      == d*z node-major -> AllGather
  L3: a3 = d * (sum zrow[src] + zrow[self]); u4 = relu(dW1p^T a3 + db1) == h3
  L4: h4' = d*(u4 dW2) -> AllGather -> spmm w/ sqd x db2 -> f (raw)
  out: xhat = d*(f dWf + s x dbf), written bf16, host-cast to f32.
"""
import sys
sys.path.insert(0, '/opt/trn_rl_repo')

import contextlib
import ctypes
import os
import types

import numpy as np
import ml_dtypes

import concourse.bacc as bacc
import concourse.bass as bass
import concourse.mybir as mybir
import concourse.tile as tile
from concourse.library_config import mlp
from concourse.vector_clock import ScopedClock
from concourse.bass_utils import run_bass_kernel_spmd


# ---- workaround: this walrus build rejects >2 sync waits on one instruction;
# spread Tile's tail-drain waits across single-wait SP NOPs.
def _patched_drain_and_barrier(self, tick_clock, wait_clock):
    nc = self.nc
    probe = nc.sync.nop()
    wait_clock.add_sem_waits(probe.ins, ScopedClock({None: tick_clock.global_clock}))
    si = probe.ins.sync_info
    waits = list(si.on_wait) if si is not None else []
    if si is not None:
        while si.on_wait:
            si.on_wait.pop()
    for w in waits:
        n = nc.sync.nop()
        n.ins.sync_info = mybir.SyncInfo(on_wait=[w], on_update=[])
    nc.sync.drain()
    nc.all_engine_barrier()
    assert self.sems is not None
    popped = nc._tile_sem_poison_stack.pop()
    assert popped is self._sem_poison
    nc.clear_and_free_semaphores(list(self.sems.allocated().values()))
    nc.all_engine_barrier()


tile.TileContext._drain_and_barrier = _patched_drain_and_barrier


# ---- optional NTFF profiling hook (GCAE_TRACE=1)
def _install_profile_hook():
    try:
        import antenv
    except ImportError:
        return False
    if getattr(antenv, "axon_hooks", None) is not None:
        return True
    so_path = "/opt/axon/libaxon_pjrt.so"
    if not os.path.exists(so_path):
        return False
    lib = ctypes.CDLL(so_path)
    if not hasattr(lib, "axon_start_nrt_profile"):
        return False
    lib.axon_start_nrt_profile.argtypes = [ctypes.POINTER(ctypes.c_int64), ctypes.c_size_t]
    lib.axon_start_nrt_profile.restype = ctypes.c_int64
    lib.axon_stop_nrt_profile.argtypes = [ctypes.c_char_p]
    lib.axon_stop_nrt_profile.restype = ctypes.c_int64

    @contextlib.contextmanager
    def _hook(output_dir, device_ids):
        import jax
        jax.devices()
        if device_ids:
            ids = (ctypes.c_int64 * len(device_ids))(*device_ids)
            rc = lib.axon_start_nrt_profile(ids, len(device_ids))
        else:
            rc = lib.axon_start_nrt_profile(None, 0)
        if rc != 0:
            raise RuntimeError(f"axon_start_nrt_profile rc={rc}")
        try:
            yield
        finally:
            n = lib.axon_stop_nrt_profile(str(output_dir).encode())
            if n < 0:
                raise RuntimeError(f"axon_stop_nrt_profile rc={n}")

    hooks = types.ModuleType("antenv.axon_hooks")
    _h = [_hook]
    hooks.set_axon_ntff_profile_hook = lambda h: _h.__setitem__(0, h)
    hooks.get_axon_ntff_profile_hook = lambda: _h[0]
    sys.modules["antenv.axon_hooks"] = hooks
    antenv.axon_hooks = hooks
    return True

F32 = mybir.dt.float32
BF16 = mybir.dt.bfloat16
I16 = mybir.dt.int16

N = 50000
NC = 8
BLK = 128
BPC = 49                 # blocks per core
NPC = BPC * BLK          # 6272 nodes per core
NPAD = NC * NPC          # 50176
HALF = NPAD // 2         # 25088
NT = BPC                 # node tiles per core
LOB = 24                 # blocks per core in sub-shard A
LOA = LOB * BLK          # 3072 rows/core in sub-shard A
HIB = NPC - LOA          # 3200 rows/core in sub-shard B


# ---------------------------------------------------------------- host prep

def preprocess(edge_index):
    src = np.asarray(edge_index[0], dtype=np.int64)
    dst = np.asarray(edge_index[1], dtype=np.int64)

    # degree includes the self-loops even though they are not in the edge
    # stream (they are applied on-device via an identity matmul)
    deg = (np.bincount(dst, minlength=N) + 1).astype(np.float64)
    dinv = np.where(deg > 0, 1.0 / np.sqrt(deg), 0.0)
    sqrtdeg = np.where(deg > 0, np.sqrt(deg), 0.0)

    # snake-deal nodes (sorted by degree desc) into 392 blocks
    nblocks = NC * BPC
    order = np.argsort(-deg, kind="stable")
    node_new = np.empty(N, dtype=np.int64)
    counts = np.zeros(nblocks, dtype=np.int64)
    bi = 0
    direction = 1
    for i, nd in enumerate(order):
        b = bi if direction == 1 else nblocks - 1 - bi
        node_new[nd] = b * BLK + counts[b]
        counts[b] += 1
        bi += 1
        if bi == nblocks:
            bi = 0
            direction = -direction
    assert counts.max() <= BLK

    # Edge halves (A/B sub-shard of the SRC) are fixed by which sub-shard a
    # node sits in; repacking nodes WITHIN a (core, sub-shard) never flips
    # any edge's half. Two host-only refinements under that invariant:
    # 1) 2D repack: redistribute nodes among their sub-shard's blocks to
    #    balance per-block (lo, hi) in-edge counts.
    # 2) slot alignment: within each sub-shard, order each core's blocks by
    #    lo-count so slot b holds similarly-sized groups on every core (the
    #    shared tile schedule takes max-over-cores per slot).
    s_p0 = node_new[src]
    d_p0 = node_new[dst]
    hf0 = ((s_p0 % NPC) >= LOA).astype(np.int64)
    node_lo = np.bincount(d_p0[hf0 == 0], minlength=NPAD).astype(np.int64)
    node_hi = np.bincount(d_p0[hf0 == 1], minlength=NPAD).astype(np.int64)

    new_pos = np.empty(NPAD, dtype=np.int64)
    for c in range(NC):
        for sh, (b0, nb) in enumerate(((0, LOB), (LOB, BPC - LOB))):
            rows = np.arange(c * NPC + b0 * BLK, c * NPC + (b0 + nb) * BLK)
            lo, hi = node_lo[rows], node_hi[rows]
            order = np.argsort(-(lo + hi), kind="stable")
            losum = np.zeros(nb); hisum = np.zeros(nb)
            nfill = np.zeros(nb, dtype=np.int64)
            assign = np.empty(len(rows), dtype=np.int64)
            for i in order:
                cost = (losum + lo[i]) ** 2 + (hisum + hi[i]) ** 2
                cost[nfill >= BLK] = np.inf
                j = int(np.argmin(cost))
                assign[i] = j
                losum[j] += lo[i]; hisum[j] += hi[i]; nfill[j] += 1
            slot_of = np.empty(nb, dtype=np.int64)
            slot_of[np.argsort(-losum, kind="stable")] = np.arange(nb)
            fill2 = np.zeros(nb, dtype=np.int64)
            for i in range(len(rows)):
                j = slot_of[assign[i]]
                new_pos[rows[i]] = c * NPC + (b0 + j) * BLK + fill2[j]
                fill2[j] += 1
    node_new = new_pos[node_new]

    s_p = node_new[src]
    d_p = node_new[dst]

    # per (core, block, half) edge lists
    core = d_p // NPC
    blk = (d_p % NPC) // BLK
    dloc = d_p % BLK
    score = s_p // NPC
    w = s_p % NPC
    hf = (w >= LOA).astype(np.int64)
    idxh = np.where(hf == 0, score * LOA + w, score * HIB + (w - LOA))

    cnt = np.zeros((NC, BPC, 2), dtype=np.int64)
    np.add.at(cnt, (core, blk, hf), 1)
    T = np.maximum(1, np.ceil(cnt.max(axis=0) / BLK).astype(np.int64))  # [BPC, 2]

    offs = np.zeros((BPC, 2), dtype=np.int64)   # tile offset of each (b, hf) group
    t = 0
    for b in range(BPC):
        for h in range(2):
            offs[b, h] = t
            t += T[b, h]
    TT = t                                       # total tiles per core

    idx_all = np.full((NC, TT * BLK), -1, dtype=np.int16)
    dloc_all = np.full((NC, TT * BLK), -1.0, dtype=np.float32)
    key = (core * BPC + blk) * 2 + hf
    ordkey = np.lexsort((idxh, key))
    ks = key[ordkey]
    sc, sb, sh = core[ordkey], blk[ordkey], hf[ordkey]
    si, sd = idxh[ordkey], dloc[ordkey]
    ne = len(ks)
    starts = np.r_[0, np.flatnonzero(np.diff(ks)) + 1]
    glen = np.diff(np.r_[starts, ne])
    pos = np.arange(ne) - np.repeat(starts, glen)
    slot = offs[sb, sh] * BLK + pos
    idx_all[sc, slot] = si.astype(np.int16)
    dloc_all[sc, slot] = sd.astype(np.float32)

    sd_pad = np.zeros(NPAD, dtype=np.float32)
    di_pad = np.zeros(NPAD, dtype=np.float32)
    sd_pad[node_new] = sqrtdeg
    di_pad[node_new] = dinv

    return dict(node_new=node_new, T=T, offs=offs, TT=TT, cnt=cnt,
                idx_all=idx_all, dloc_all=dloc_all,
                sqrtdeg=sd_pad, dinv=di_pad)


def make_inmaps(pre, x, weights):
    """weights: dict of padded bf16 weight/bias arrays (shared across cores)."""
    node_new = pre["node_new"]
    TT = pre["TT"]
    Tmax = int(pre["T"].max())
    bf = ml_dtypes.bfloat16

    # host-staged, dinv-prescaled node-major x (the L1 gather source)
    xs = np.zeros((NPAD, 128), dtype=np.float32)
    xs[node_new] = np.asarray(x, dtype=np.float32)
    xs *= pre["dinv"][:, None]
    xs3 = xs.reshape(NC, NPC, 128)
    xsA = np.ascontiguousarray(xs3[:, :LOA, :].reshape(NC * LOA, 128)).astype(bf)
    xsB = np.ascontiguousarray(xs3[:, LOA:, :].reshape(NC * HIB, 128)).astype(bf)

    in_maps = []
    for c in range(NC):
        m = {}
        m["xsA"] = xsA
        m["xsB"] = xsB
        m["xloc"] = np.ascontiguousarray(xs3[c]).astype(bf)    # [NPC, 128]
        idx = pre["idx_all"][c]
        m["idxs"] = np.tile(idx.reshape(TT * 8, 16).T, (8, 1)).copy()
        dl = pre["dloc_all"][c].reshape(TT, BLK).T             # [128, TT]
        m["dstloc"] = np.ascontiguousarray(dl, dtype=bf)
        sl = slice(c * NPC, (c + 1) * NPC)
        gc = pre["cnt"][c].T.reshape(1, 2 * BPC)               # [1, 2*BPC], hf-major
        m["gcount"] = np.ascontiguousarray(gc, dtype=np.int32)
        m["sqrtdeg_row"] = pre["sqrtdeg"][sl][None, :].astype(bf)
        m["dinv_col"] = pre["dinv"][sl].reshape(BPC, BLK).T.astype(np.float32).copy()
        m["dinv2_col"] = (pre["dinv"][sl] ** 2).reshape(BPC, BLK).T.astype(np.float32).copy()
        m["dinvb"] = np.tile(pre["dinv"][sl][None, :], (128, 1)).astype(bf)
        m["one_row"] = np.ones((1, 128), dtype=np.float32).astype(bf)
        R = np.tile(np.arange(BLK, dtype=np.float32), (128, Tmax)).astype(bf)
        m["Rbig"] = R
        m["ident"] = np.eye(128, dtype=np.float32).astype(bf)
        m.update(weights)
        in_maps.append(m)
    return in_maps


def pad_weights(eW1, eb1, eW2, eb2, eWf, ebf, dW1, db1, dW2, db2, dWf, dbf):
    bf = ml_dtypes.bfloat16
    w = {}
    w["eW1"] = np.asarray(eW1, np.float32).astype(bf)                       # [128,128]
    eW2p = np.zeros((128, 128), np.float32); eW2p[:, :64] = eW2
    w["eW2p"] = eW2p.astype(bf)
    eWfp = np.zeros((128, 128), np.float32); eWfp[:64, :64] = eWf
    w["eWfp"] = eWfp.astype(bf)                                             # [128,128]
    dW1p = np.zeros((128, 256), np.float32); dW1p[:64] = dW1
    w["dW1p"] = dW1p.astype(bf)                                             # [128,256]
    w["dW2"] = np.asarray(dW2, np.float32).astype(bf)                       # [256,128]
    w["dWf"] = np.asarray(dWf, np.float32).astype(bf)                       # [128,1024]
    w["eb1_row"] = np.asarray(eb1, np.float32)[None, :].astype(bf)          # [1,128]
    eb2r = np.zeros((1, 128), np.float32); eb2r[0, :64] = eb2
    w["eb2p_row"] = eb2r.astype(bf)
    ebfr = np.zeros((1, 128), np.float32); ebfr[0, :64] = ebf
    w["ebf_row"] = ebfr.astype(bf)                                          # [1,128]
    w["db1_row"] = np.asarray(db1, np.float32)[None, :].astype(bf)          # [1,256]
    w["db2_row"] = np.asarray(db2, np.float32)[None, :].astype(bf)          # [1,128]
    w["dbf_row"] = np.asarray(dbf, np.float32)[None, :].astype(bf)          # [1,1024]
    return w


# ---------------------------------------------------------------- device program

def build_program(T, offs, TT):
    """T: [BPC,2] tiles per (block,half); offs: [BPC,2] tile offsets; TT total tiles."""
    Tmax = int(T.max())
    nc = bacc.Bacc(None, target_bir_lowering=False, num_swdge_queues=4)

    # ---- I/O
    xsA_d = nc.dram_tensor("xsA", [NC * LOA, 128], BF16, kind="ExternalInput")
    xsB_d = nc.dram_tensor("xsB", [NC * HIB, 128], BF16, kind="ExternalInput")
    xloc_d = nc.dram_tensor("xloc", [NPC, 128], BF16, kind="ExternalInput")
    idx_d = nc.dram_tensor("idxs", [128, TT * 8], I16, kind="ExternalInput")
    dloc_d = nc.dram_tensor("dstloc", [128, TT], BF16, kind="ExternalInput")
    gcount_d = nc.dram_tensor("gcount", [1, 2 * BPC], mybir.dt.int32, kind="ExternalInput")
    sqd_d = nc.dram_tensor("sqrtdeg_row", [1, NPC], BF16, kind="ExternalInput")
    dinv_d = nc.dram_tensor("dinv_col", [128, BPC], F32, kind="ExternalInput")
    dinv2_d = nc.dram_tensor("dinv2_col", [128, BPC], F32, kind="ExternalInput")
    dinvb_d = nc.dram_tensor("dinvb", [128, NPC], BF16, kind="ExternalInput")
    one_d = nc.dram_tensor("one_row", [1, 128], BF16, kind="ExternalInput")
    R_d = nc.dram_tensor("Rbig", [128, Tmax * 128], BF16, kind="ExternalInput")
    id_d = nc.dram_tensor("ident", [128, 128], BF16, kind="ExternalInput")
    wnames = {"eW1": [128, 128], "eW2p": [128, 128], "eWfp": [128, 128],
              "dW1p": [128, 256], "dW2": [256, 128], "dWf": [128, 1024],
              "eb1_row": [1, 128], "eb2p_row": [1, 128], "ebf_row": [1, 128],
              "db1_row": [1, 256], "db2_row": [1, 128], "dbf_row": [1, 1024]}
    w_d = {k: nc.dram_tensor(k, shp, BF16, kind="ExternalInput")
           for k, shp in wnames.items()}
    out_d = nc.dram_tensor("xhat", [2, NPC, 512], BF16, kind="ExternalOutput")

    with tile.TileContext(nc) as tc:
        with tc.tile_pool(name="const", bufs=1) as cpool, \
             tc.tile_pool(name="acts", bufs=1) as apool, \
             tc.tile_pool(name="dram", bufs=1, space="DRAM") as dram, \
             tc.tile_pool(name="wps", bufs=4, space="PSUM") as pps, \
             tc.tile_pool(name="wtr", bufs=4, space="PSUM") as ptr, \
             tc.tile_pool(name="wm", bufs=8) as pm, \
             tc.tile_pool(name="ws", bufs=4) as psl, \
             tc.tile_pool(name="wh", bufs=3) as ph, \
             tc.tile_pool(name="wn", bufs=4) as phn:
            nc.gpsimd.load_library(mlp)

            # ---- persistent SBUF state (gather-critical loads first: the
            # sync engine issues DMAs in order and the first dma_gather only
            # needs gcount + idx)
            gcount_sb = cpool.tile([1, 2 * BPC], mybir.dt.int32, name="gcount_sb")
            nc.sync.dma_start(gcount_sb[:], gcount_d[:])
            idx_sb = cpool.tile([128, TT * 8], I16, name="idx_sb")
            nc.sync.dma_start(idx_sb[:], idx_d[:])
            dloc_sb = cpool.tile([128, TT], BF16, name="dloc_sb")
            nc.sync.dma_start(dloc_sb[:], dloc_d[:])
            R_sb = cpool.tile([128, Tmax * 128], BF16, name="R_sb")
            nc.sync.dma_start(R_sb[:], R_d[:])
            id_sb = cpool.tile([128, 128], BF16, name="id_sb")
            nc.sync.dma_start(id_sb[:], id_d[:])
            w_sb = {}
            for k, shp in wnames.items():
                if shp[0] > 128:
                    continue
                t = cpool.tile(shp, BF16, name=f"w_{k}")
                nc.sync.dma_start(t[:], w_d[k][:])
                w_sb[k] = t
            dW2a = cpool.tile([128, 128], BF16, name="w_dW2a")
            nc.sync.dma_start(dW2a[:], w_d["dW2"][0:128, :])
            dW2b = cpool.tile([128, 128], BF16, name="w_dW2b")
            nc.sync.dma_start(dW2b[:], w_d["dW2"][128:256, :])
            sqd_sb = cpool.tile([1, NPC], BF16, name="sqd_sb")
            nc.sync.dma_start(sqd_sb[:], sqd_d[:])
            dinv_sb = cpool.tile([128, BPC], F32, name="dinv_sb")
            nc.sync.dma_start(dinv_sb[:], dinv_d[:])
            dinv2_sb = cpool.tile([128, BPC], F32, name="dinv2_sb")
            nc.sync.dma_start(dinv2_sb[:], dinv2_d[:])
            dinvb_sb = cpool.tile([128, NPC], BF16, name="dinvb_sb")
            nc.sync.dma_start(dinvb_sb[:], dinvb_d[:])
            one_sb = cpool.tile([1, 128], BF16, name="one_sb")
            nc.sync.dma_start(one_sb[:], one_d[:])

            R3 = R_sb[:].rearrange("p (t d) -> p t d", d=128)

            uT = {}  # feature-major activation arrays per stage
            for nm in ("a1", "u2", "w2", "a3", "u4a", "u4b", "f"):
                uT[nm] = apool.tile([128, NPC], BF16, name=f"{nm}T")

            qstate = [0]

            # ---------------- SpMM: gather-aggregate into per-block PSUM
            # chains; pass A includes the self-loop identity mm (+ optional
            # dst-side sqd x bias mm); pass B re-injects the bf16 pass-A
            # partial via an identity matmul; per-block copy-out via copy_cb.
            def spmm(layer, bufA, bufB, locA, locB, bias_row, out_t, copy_cb,
                     epi=None, border=None):
                for hf in range(2):
                    buf = bufA if hf == 0 else bufB
                    blks = range(BPC) if (hf == 0 or border is None) else border
                    for b in blks:
                        Tb = int(T[b, hf]); off = int(offs[b, hf])
                        pb = pps.tile([128, 128], F32, tag="pb")
                        if hf == 0:
                            hblk = ph.tile([128, 128], BF16, tag="hblk")
                            if b < LOB:
                                nc.sync.dma_start(hblk[:], locA[b * 128:(b + 1) * 128, :])
                            else:
                                nc.sync.dma_start(hblk[:], locB[(b - LOB) * 128:(b - LOB + 1) * 128, :])
                            if bias_row is not None:
                                nc.tensor.matmul(
                                    pb[:], bias_row[0:1, :],
                                    sqd_sb[0:1, b * 128:(b + 1) * 128],
                                    start=True, stop=False)
                            nc.tensor.matmul(
                                pb[:], hblk[:], id_sb[:],
                                start=(bias_row is None), stop=False)
                        else:
                            nc.tensor.matmul(
                                pb[:], id_sb[:], out_t[:, b * 128:(b + 1) * 128],
                                start=True, stop=False)
                        msg = pm.tile([128, Tmax, 128], BF16, tag="msg")
                        if layer == 1 and hf == 0 and b < 8:
                            nc.vector.memset(msg[:], 0.0)
                        creg = nc.gpsimd.alloc_register()
                        nc.gpsimd.load(creg, gcount_sb[0:1, hf * BPC + b:hf * BPC + b + 1])
                        nc.gpsimd.dma_gather(
                            msg[:, :Tb, :], buf[:],
                            idx_sb[:, off * 8:(off + Tb) * 8],
                            Tb * 128, creg, 128, single_packet=False,
                            queue_num=qstate[0])
                        qstate[0] = (qstate[0] + 1) % 4
                        S = psl.tile([128, Tmax, 128], BF16, tag="S")
                        d3 = dloc_sb[:, off:off + Tb].broadcast_to([128, Tb, 128])
                        nc.vector.tensor_tensor(S[:, :Tb, :], R3[:, :Tb, :], d3,
                                                mybir.AluOpType.is_equal)
                        for t in range(Tb):
                            nc.tensor.matmul(
                                pb[:], msg[:, t, :], S[:, t, :],
                                start=False, stop=(t == Tb - 1))
                        osl = out_t[:, b * 128:(b + 1) * 128]
                        copy_cb(b, hf, pb, osl)
                        if epi is not None and hf == 1:
                            epi(b)

            def cb_plain(b, hf, pb, osl):
                nc.scalar.activation(osl, pb[:], mybir.ActivationFunctionType.Copy)

            def cb_dinvb(b, hf, pb, osl):
                if hf == 0:
                    nc.scalar.activation(osl, pb[:],
                                         mybir.ActivationFunctionType.Copy)
                else:
                    nc.vector.tensor_tensor(
                        osl, pb[:], dinvb_sb[:, b * 128:(b + 1) * 128],
                        mybir.AluOpType.mult)

            def cb_final(b, hf, pb, osl):
                nc.scalar.activation(osl, pb[:], mybir.ActivationFunctionType.Copy)
                if hf == 0:
                    return
                # final stage for block b: xhat = d*(f dWf + s x dbf), bf16 out
                for cb in range(2):
                    pf = ptr.tile([128, 512], F32, tag="tr")
                    nc.tensor.matmul(pf[:], osl,
                                     w_sb["dWf"][:, cb * 512:(cb + 1) * 512],
                                     start=True, stop=False)
                    nc.tensor.matmul(pf[:],
                                     sqd_sb[0:1, b * 128:(b + 1) * 128],
                                     w_sb["dbf_row"][0:1, cb * 512:(cb + 1) * 512],
                                     start=False, stop=True)
                    ob = phn.tile([128, 512], BF16, tag="ob")
                    nc.scalar.activation(ob[:], pf[:],
                                         mybir.ActivationFunctionType.Copy,
                                         scale=dinv_sb[:, b:b + 1])
                    nc.sync.dma_start(out_d[cb, b * 128:(b + 1) * 128, :], ob[:])

            # ---------------- feature-major "lite" transform, one 128-node tile:
            # out_fm = act(W^T @ in_fm + bias x ones)
            def tlite_tile(nt, in_t, Ws, bias_row, out_ts, act):
                for chb in range(len(out_ts)):
                    pt = ptr.tile([128, 128], F32, tag="tr")
                    nc.tensor.matmul(pt[:], Ws[:, chb * 128:(chb + 1) * 128],
                                     in_t[:, nt * 128:(nt + 1) * 128],
                                     start=True, stop=False)
                    nc.tensor.matmul(pt[:], bias_row[0:1, chb * 128:(chb + 1) * 128],
                                     one_sb[0:1, :], start=False, stop=True)
                    nc.scalar.activation(
                        out_ts[chb][:, nt * 128:(nt + 1) * 128], pt[:], act)

            # ---------------- node-major transform + shard write, one tile:
            # shard rows = scale_col * (sum_k u_k^T @ W_k [+ s x bias])
            def transform_tile(nt, parts, bias_row, shards, scale_col):
                shA, shB = shards
                hb = ptr.tile([128, 128], F32, tag="tr")
                for ki, (ut, Wk) in enumerate(parts):
                    nc.tensor.matmul(hb[:], ut[:, nt * 128:(nt + 1) * 128],
                                     Wk[:], start=(ki == 0),
                                     stop=(bias_row is None and
                                           ki == len(parts) - 1))
                if bias_row is not None:
                    nc.tensor.matmul(hb[:], sqd_sb[0:1, nt * 128:(nt + 1) * 128],
                                     bias_row[0:1, :], start=False, stop=True)
                hn = phn.tile([128, 128], BF16, tag="hn")
                nc.scalar.activation(hn[:], hb[:],
                                     mybir.ActivationFunctionType.Copy,
                                     scale=scale_col[:, nt:nt + 1])
                if nt < LOB:
                    nc.sync.dma_start(shA[nt * 128:(nt + 1) * 128, :], hn[:])
                else:
                    nc.sync.dma_start(shB[(nt - LOB) * 128:(nt - LOB + 1) * 128, :], hn[:])

            def mkshard(name, ch):
                sA = dram.tile([LOA, ch], BF16, name=f"{name}_shardA")
                sB = dram.tile([HIB, ch], BF16, name=f"{name}_shardB")
                fA = dram.tile([NC * LOA, ch], BF16, name=f"{name}_fullA", addr_space="Shared")
                fB = dram.tile([NC * HIB, ch], BF16, name=f"{name}_fullB", addr_space="Shared")
                return sA, sB, fA, fB

            def allgather(sX, fX):
                nc.gpsimd.collective_compute(
                    "AllGather", mybir.AluOpType.bypass,
                    replica_groups=[list(range(NC))],
                    ins=[sX.opt()], outs=[fX.opt()])

            # ================= network =================
            h2sA, h2sB, h2fA, h2fB = mkshard("h2", 128)
            h3sA, h3sB, h3fA, h3fB = mkshard("h3", 128)
            h4sA, h4sB, h4fA, h4fB = mkshard("h4", 128)

            relu_act = mybir.ActivationFunctionType.Relu

            # L1: gather host-staged xs; a1 = d*(sum + self); per-block
            # epilogue: u2 tile = relu(eW1^T a1 + eb1) == h1, then
            # h2' tile = d*(u2 eW2p) -> shard
            def epi1(b):
                tlite_tile(b, uT["a1"], w_sb["eW1"], w_sb["eb1_row"],
                           [uT["u2"]], relu_act)
                transform_tile(b, [(uT["u2"], w_sb["eW2p"])], None,
                               (h2sA[:], h2sB[:]), dinv_sb)

            spmm(1, xsA_d, xsB_d, xloc_d[0:LOA, :], xloc_d[LOA:NPC, :],
                 None, uT["a1"], cb_dinvb, epi=epi1)
            allgather(h2sA, h2fA)
            allgather(h2sB, h2fB)

            # L2 spmm (+eb2p); epilogue: zrow tile = d^2*(w2 eWfp + s x ebf)
            def epi2(b):
                transform_tile(b, [(uT["w2"], w_sb["eWfp"])], w_sb["ebf_row"],
                               (h3sA[:], h3sB[:]), dinv2_sb)

            spmm(2, h2fA, h2fB, h2sA[:], h2sB[:], w_sb["eb2p_row"], uT["w2"],
                 cb_plain, epi=epi2)
            allgather(h3sA, h3fA)
            allgather(h3sB, h3fB)

            # L3: a3 = d*(sum zrow + self); epilogue: u4 = relu(dW1p^T a3 +
            # db1) == h3, then h4' tile = d*(u4 dW2) -> shard
            def epi3(b):
                tlite_tile(b, uT["a3"], w_sb["dW1p"], w_sb["db1_row"],
                           [uT["u4a"], uT["u4b"]], relu_act)
                transform_tile(b, [(uT["u4a"], dW2a), (uT["u4b"], dW2b)], None,
                               (h4sA[:], h4sB[:]), dinv_sb)

            spmm(3, h3fA, h3fB, h3sA[:], h3sB[:], None, uT["a3"], cb_dinvb,
                 epi=epi3)
            allgather(h4sA, h4fA)
            allgather(h4sB, h4fB)

            # L4 spmm (+db2); final stage emitted per block via cb_final.
            # B-pass in descending group size so the post-stream tail chain
            # (last gather -> matmuls -> final -> DMA) is minimal.
            l4order = sorted(range(BPC), key=lambda b: -int(T[b, 1]))
            spmm(4, h4fA, h4fB, h4sA[:], h4sB[:], w_sb["db2_row"], uT["f"],
                 cb_final, border=l4order)

    nc.finalize()
    return nc


# ---------------------------------------------------------------- entry point

def kernel(x, edge_index, eW1, eb1, eW2, eb2, eWf, ebf,
           dW1, db1, dW2, db2, dWf, dbf):
    x = np.asarray(x, dtype=np.float32)
    edge_index = np.asarray(edge_index)

    pre = preprocess(edge_index)
    w = pad_weights(eW1, eb1, eW2, eb2, eWf, ebf, dW1, db1, dW2, db2, dWf, dbf)
    in_maps = make_inmaps(pre, x, w)
    nc = build_program(pre["T"], pre["offs"], pre["TT"])

    trace = os.environ.get("GCAE_TRACE", "0") == "1"
    if trace:
        trace = _install_profile_hook()
    res = None
    last_err = None
    for attempt in range(3):
        try:
            res = run_bass_kernel_spmd(nc, in_maps, core_ids=list(range(NC)),
                                       trace=trace and attempt == 0)
            break
        except Exception as e:  # transient device wedge: retry, drop tracing
            last_err = e
    if res is None:
        raise last_err
    if trace and res.exec_time_ns:
        print(f"HW exec time: {res.exec_time_ns} ns")

    xhat_pad = np.empty((NPAD, 1024), dtype=np.float32)
    for c in range(NC):
        o = np.asarray(res.results[c]["xhat"]).astype(np.float32)
        xhat_pad[c * NPC:(c + 1) * NPC, 0:512] = o[0]
        xhat_pad[c * NPC:(c + 1) * NPC, 512:1024] = o[1]
    return xhat_pad[pre["node_new"]]
